# revision 1
# baseline (speedup 1.0000x reference)
"""Trainium2 Bass kernel for nn_DotAttention (B=4, Tq=Tv=2048, D=1024, 16 heads).

Sharding: core c -> (batch b = c//2, head-group hg = c%2 of 8 heads).
Each core computes q/k/v projections for its 512 att-dim slice, masked
softmax attention in transposed-energy layout, and a partial final
projection with its 512-row slice of Wf. Host sums the two partials per
batch and adds the bias constant (bv @ Wf + bf, exact because attention
weights sum to 1).

Layouts (SBUF is [128 partitions, free]):
  qT/kT  [128, 4, T]   partition+chunk = att-dim slice c, free = time
  v      [128, NJ, 520] partition = Tv tile, per head 65 cols (64 v + ones)
  energy^T in PSUM [128(Tv), 2*512] both heads of a pair side by side
  ctx^T  in PSUM [65, 512] per head; row 64 = softmax denominator (ones col)

All matmuls run in float32r (TF32-like, full PE rate at N>=256).
The program is specialized on NJ = ceil(max(value_lens)/128): fully
masked Tv chunks beyond that are skipped; per-core masking is handled by
a per-partition additive bias (-1e30) on the exp activation.
"""

import sys

sys.path.insert(0, "/opt/trn_rl_repo")

import numpy as np

import concourse.bacc as bacc
import concourse.tile as tile
import concourse.mybir as mybir
from concourse.bass_utils import run_bass_kernel_spmd

F32 = mybir.dt.float32
F32R = mybir.dt.float32r
BF16 = mybir.dt.bfloat16
F16 = mybir.dt.float16
MMDT = F32R
AF = mybir.ActivationFunctionType

B, T, D, ATT = 4, 2048, 1024, 1024
NH, DH = 16, 64
HPC = 8  # heads per core
CD = 512  # att-dim slice per core
NCORES = 8
LARGE = 1e30
SW = 512  # time-span width per streamed input chunk

_cache = {}


def build_nc(NJ, phases="ABC", loop_n=1, mmdt=None, splice=True,
             ebufs=2, pcybufs=4, exbufs=3, ehalf=False, pebc=False,
             qkf16=False):
    global MMDT
    if mmdt is not None:
        MMDT = mmdt
    key = (NJ, phases, loop_n, str(MMDT), splice, ebufs, pcybufs, exbufs, ehalf, pebc, qkf16)
    QKDT = F16 if qkf16 else MMDT
    if key in _cache:
        return _cache[key]
    NSV = (NJ * 128 + SW - 1) // SW  # spans of Tv needed for k/v
    TV = NSV * SW  # padded Tv extent materialized for kT
    nc = bacc.Bacc("TRN2", target_bir_lowering=False, debug=False, num_devices=NCORES)

    xq_d = nc.dram_tensor("xq", [D, T], MMDT, kind="ExternalInput")  # query[b].T
    xv_d = nc.dram_tensor("xv", [D, T], MMDT, kind="ExternalInput")  # value[b].T
    wq_d = nc.dram_tensor("wq", [D, CD], MMDT, kind="ExternalInput")
    wk_d = nc.dram_tensor("wk", [D, CD], MMDT, kind="ExternalInput")
    wv_d = nc.dram_tensor("wv", [D, HPC * 65], MMDT, kind="ExternalInput")
    wf_d = nc.dram_tensor("wf", [CD, ATT], MMDT, kind="ExternalInput")
    mask_d = nc.dram_tensor("mask", [128, NJ], F32, kind="ExternalInput")
    bq_d = nc.dram_tensor("bqc", [128, 4], F32, kind="ExternalInput")
    bk_d = nc.dram_tensor("bkc", [128, 4], F32, kind="ExternalInput")
    y_d = nc.dram_tensor("y", [T, ATT], F32, kind="ExternalOutput")

    xq_r = xq_d[:, :].rearrange("(kc p) n -> p kc n", p=128)  # [128, 8, T]
    xv_r = xv_d[:, :].rearrange("(kc p) n -> p kc n", p=128)
    wq_r = wq_d[:, :].rearrange("(kc p) m -> p kc m", p=128)  # [128, 8, 512]
    wk_r = wk_d[:, :].rearrange("(kc p) m -> p kc m", p=128)
    wv_r = wv_d[:, :].rearrange("(kc p) m -> p kc m", p=128)  # [128, 8, 520]
    wf_r = wf_d[:, :].rearrange("(kc p) n -> p kc n", p=128)  # [128, 4, 1024]

    with tile.TileContext(nc) as tc:
        from contextlib import ExitStack
        _st = ExitStack()
        if loop_n > 1:
            _st.enter_context(tc.For_i(0, loop_n, 1))
        with _st, tc.tile_pool(name="persist", bufs=1) as persist:
            qT = persist.tile([128, 4, T], QKDT)
            kT = persist.tile([128, 4, TV], QKDT)
            v = persist.tile([128, NJ, HPC * 65], MMDT)
            mask = persist.tile([128, NJ], F32)
            bqc = persist.tile([128, 4], F32)
            bkc = persist.tile([128, 4], F32)
            ones = persist.tile([1, 64], MMDT)

            def set_ones(dst, src):
                nc.scalar.activation(out=dst, in_=src, func=AF.Identity,
                                     bias=1.0, scale=0.0)

            # ---------------- Phase A: projections ----------------
            with (
                tc.tile_pool(name="wpool", bufs=1) as wpool,
                tc.tile_pool(name="chunks", bufs=2 if NJ >= 15 else 3) as chunks,
                tc.tile_pool(name="ppq", bufs=4, space="PSUM") as ppq,
                tc.tile_pool(name="ppv", bufs=2, space="PSUM") as ppv,
            ):
                wq = wpool.tile([128, 8, CD], MMDT)
                wk = wpool.tile([128, 8, CD], MMDT)
                wv = wpool.tile([128, 8, HPC * 65], MMDT)
                for s in (range(T // SW) if ("A" in phases or "D" in phases)
                          else []):
                    sl = slice(s * SW, (s + 1) * SW)
                    if s < NSV:
                        xv_c = chunks.tile([128, 8, SW], MMDT, tag="xc")
                        nc.sync.dma_start(out=xv_c, in_=xv_r[:, :, sl])
                        if s == 0:
                            # wk/wv split per contraction chunk so the first
                            # projection matmuls start as soon as chunk 0 lands
                            for kc in range(8):
                                nc.sync.dma_start(out=wk[:, kc, :],
                                                  in_=wk_r[:, kc, :])
                            for kc in range(8):
                                nc.sync.dma_start(out=wv[:, kc, :],
                                                  in_=wv_r[:, kc, :])
                            nc.sync.dma_start(out=mask, in_=mask_d[:, :])
                            nc.sync.dma_start(out=bqc, in_=bq_d[:, :])
                            nc.sync.dma_start(out=bkc, in_=bk_d[:, :])
                            for kc in range(8):
                                nc.scalar.dma_start(out=wq[:, kc, :],
                                                    in_=wq_r[:, kc, :])
                        # kT columns for this span
                        for m in (range(4) if "A" in phases else []):
                            ps = ppq.tile([128, SW], F32, tag="qk")
                            for kc in range(8):
                                nc.tensor.matmul(
                                    ps[:, :],
                                    lhsT=wk[:, kc, m * 128:(m + 1) * 128],
                                    rhs=xv_c[:, kc, :],
                                    start=(kc == 0), stop=(kc == 7),
                                )
                            with nc.allow_low_precision(reason="qk store"):
                                nc.vector.tensor_scalar_add(
                                    kT[:, m, sl], ps[:, :], bkc[:, m:m + 1])
                        # v rows for this span (Tv tiles of 128)
                        for jt in (range(SW // 128) if "A" in phases else []):
                            j = s * (SW // 128) + jt
                            if j >= NJ:
                                continue
                            ps = ppv.tile([128, HPC * 65], F32, tag="v")
                            for kc in range(8):
                                nc.tensor.matmul(
                                    ps[:, 0:512],
                                    lhsT=xv_c[:, kc, jt * 128:(jt + 1) * 128],
                                    rhs=wv[:, kc, 0:512],
                                    start=(kc == 0), stop=(kc == 7),
                                )
                                nc.tensor.matmul(
                                    ps[:, 512:520],
                                    lhsT=xv_c[:, kc, jt * 128:(jt + 1) * 128],
                                    rhs=wv[:, kc, 512:520],
                                    start=(kc == 0), stop=(kc == 7),
                                )
                            nc.vector.tensor_copy(out=v[:, j, :], in_=ps[:, :])
                            vj = v[:, j, :].rearrange("p (h x) -> p h x", x=65)
                            set_ones(vj[:, :, 64:65], vj[:, :, 64:65])
                    # qT columns for this span
                    xq_c = chunks.tile([128, 8, SW], MMDT, tag="xc")
                    nc.scalar.dma_start(out=xq_c, in_=xq_r[:, :, sl])
                    for m in (range(4) if "A" in phases else []):
                        ps = ppq.tile([128, SW], F32, tag="qk")
                        for kc in range(8):
                            nc.tensor.matmul(
                                ps[:, :],
                                lhsT=wq[:, kc, m * 128:(m + 1) * 128],
                                rhs=xq_c[:, kc, :],
                                start=(kc == 0), stop=(kc == 7),
                            )
                        with nc.allow_low_precision(reason="qk store"):
                            nc.vector.tensor_scalar_add(
                                qT[:, m, sl], ps[:, :], bqc[:, m:m + 1])

            # ---------------- Phase B: attention ----------------
            with tc.tile_pool(name="bc_sbuf", bufs=1) as bcp:
                ctxT = bcp.tile([128, 4, T], MMDT)
                wf = bcp.tile([128, 4, ATT], MMDT)
                nc.scalar.dma_start(out=wf, in_=wf_r)
                if "Z" in phases:  # timing probe: fill ctxT without attention
                    for kc in range(4):
                        for cc in range(4):
                            set_ones(ctxT[:, kc, cc * 512:(cc + 1) * 512],
                                     ctxT[:, kc, cc * 512:(cc + 1) * 512])
                with (
                    tc.tile_pool(name="expp", bufs=exbufs) as expp,
                    tc.tile_pool(name="workp", bufs=4) as workp,
                    tc.tile_pool(name="yp", bufs=4) as yp,
                    tc.tile_pool(name="rsd", bufs=4, space="DRAM") as rsd,
                    tc.tile_pool(name="pe", bufs=ebufs, space="PSUM") as pe_pool,
                    tc.tile_pool(name="pcy", bufs=pcybufs, space="PSUM") as pcy,
                ):
                    def emit_c_unit(i, n):
                        y_ps = pcy.tile([128, 512], F32, tag="cy",
                                        name=f"y_{i}_{n}")
                        for kc in range(4):
                            nc.tensor.matmul(
                                y_ps[:, :],
                                lhsT=ctxT[:, kc, i * 128:(i + 1) * 128],
                                rhs=wf[:, kc, n * 512:(n + 1) * 512],
                                start=(kc == 0), stop=(kc == 3),
                            )
                        y_sb = yp.tile([128, 512], F32, tag="ysb")
                        nc.vector.tensor_copy(out=y_sb[:, :], in_=y_ps[:, :])
                        nc.scalar.dma_start(
                            out=y_d[i * 128:(i + 1) * 128,
                                    n * 512:(n + 1) * 512],
                            in_=y_sb[:, :])

                    # C units for block ib-1 are spliced into block ib's
                    # ACT-bound attention to fill PE idle slots
                    pending = []
                    for ib in range(4):  # Tq block of 512
                        ibs = slice(ib * 512, (ib + 1) * 512)
                        for hp in (range(4) if "B" in phases else []):
                            ctxA = pcy.tile([65, 512], F32, tag="cy")
                            ctxB = pcy.tile([65, 512], F32, tag="cy")
                            ctx_ps = (ctxA[:, :], ctxB[:, :])
                            for j in range(NJ):
                                e_ps = pe_pool.tile([128, 1024], F32, tag="e")
                                for hh in range(2):
                                    p0 = hh * 64
                                    nc.tensor.matmul(
                                        e_ps[:, hh * 512:(hh + 1) * 512],
                                        lhsT=kT[p0:p0 + 64, hp,
                                                j * 128:(j + 1) * 128],
                                        rhs=qT[p0:p0 + 64, hp, ibs],
                                        start=True, stop=True,
                                    )
                                ex = expp.tile([128, 1024], MMDT, tag="ex")
                                nc.scalar.activation(out=ex[:, :], in_=e_ps[:, :],
                                                     func=AF.Exp,
                                                     bias=mask[:, j:j + 1],
                                                     scale=1.0)
                                for hh in range(2):
                                    h = hp * 2 + hh
                                    nc.tensor.matmul(
                                        ctx_ps[hh],
                                        lhsT=v[:, j, h * 65:(h + 1) * 65],
                                        rhs=ex[:, hh * 512:(hh + 1) * 512],
                                        start=(j == 0), stop=(j == NJ - 1),
                                    )
                            for hh in range(2):
                                p0 = hh * 64
                                rs = workp.tile([1, 512], F32, tag="rs")
                                nc.vector.reciprocal(out=rs[:, :],
                                                     in_=ctx_ps[hh][64:65, :])
                                rs_dr = rsd.tile([1, 512], F32, tag="rsd")
                                nc.sync.dma_start(out=rs_dr[:, :], in_=rs[:, :])
                                bc_sb = workp.tile([64, 512], F32, tag="bcs")
                                nc.sync.dma_start(
                                    out=bc_sb[:, :],
                                    in_=rs_dr[0:1, :].partition_broadcast(64))
                                nc.vector.tensor_mul(
                                    ctxT[p0:p0 + 64, hp, ibs],
                                    ctx_ps[hh][0:64, :], bc_sb[:, :],
                                )
                            for _ in range(2):
                                if pending:
                                    emit_c_unit(*pending.pop(0))
                        while pending:
                            emit_c_unit(*pending.pop(0))
                        if "C" in phases:
                            pending = [(i, n) for i in range(ib * 4, ib * 4 + 4)
                                       for n in range(2)]
                    while pending:
                        emit_c_unit(*pending.pop(0))
    nc.compile()
    _cache[key] = nc
    return nc


def make_in_maps(query, value, value_lens, Wq, bq, Wk, bk, Wv, bv, Wf, bf,
                 mm_np=np.float32):
    query = np.ascontiguousarray(np.asarray(query, np.float32))
    value = np.ascontiguousarray(np.asarray(value, np.float32))
    value_lens = np.asarray(value_lens)
    Wq = np.asarray(Wq, np.float32)
    Wk = np.asarray(Wk, np.float32)
    Wv = np.asarray(Wv, np.float32)
    Wf = np.asarray(Wf, np.float32)
    bq = np.asarray(bq, np.float32)
    bk = np.asarray(bk, np.float32)

    scale = 1.0 / np.sqrt(np.float32(DH))
    effL = [int(l) if l > 0 else T for l in value_lens]
    NJ = max(1, int(np.ceil(max(effL) / 128)))

    in_maps = []
    for c in range(NCORES):
        b, hg = c // 2, c % 2
        L = int(value_lens[b])
        cs = slice(hg * CD, (hg + 1) * CD)
        xq = query[b].T.copy()
        if L == 0:
            xq = np.zeros_like(xq)
        xv = value[b].T.copy()
        wq = (Wq[:, cs] * scale).copy()
        wk = Wk[:, cs].copy()
        wv = np.zeros((D, HPC * 65), np.float32)
        for h in range(HPC):
            wv[:, h * 65:h * 65 + 64] = Wv[:, hg * CD + h * 64:hg * CD + (h + 1) * 64]
        wf = Wf[cs, :].copy()
        mask = np.zeros((128, NJ), np.float32)
        if L > 0:
            idx = np.arange(NJ * 128).reshape(NJ, 128).T  # [128, NJ]
            mask[idx >= L] = -LARGE
        bqc = (bq[cs] * scale).reshape(4, 128).T.copy()
        bkc = bk[cs].reshape(4, 128).T.copy()
        in_maps.append({
            "xq": xq.astype(mm_np), "xv": xv.astype(mm_np),
            "wq": wq.astype(mm_np), "wk": wk.astype(mm_np),
            "wv": wv.astype(mm_np), "wf": wf.astype(mm_np),
            "mask": mask, "bqc": bqc, "bkc": bkc,
        })
    return in_maps, NJ


def assemble(results, Wv, bv, Wf, bf):
    Wv = np.asarray(Wv, np.float32)
    bv = np.asarray(bv, np.float32)
    Wf = np.asarray(Wf, np.float32)
    bf = np.asarray(bf, np.float32)
    out = np.empty((B, T, ATT), np.float32)
    const = (bv @ Wf + bf).astype(np.float32)
    for b in range(B):
        out[b] = results[2 * b]["y"] + results[2 * b + 1]["y"] + const
    return out


def kernel(query, value, value_lens, Wq, bq, Wk, bk, Wv, bv, Wf, bf):
    in_maps, NJ = make_in_maps(query, value, value_lens, Wq, bq, Wk, bk,
                               Wv, bv, Wf, bf)
    nc = build_nc(NJ)
    res = run_bass_kernel_spmd(nc, in_maps, list(range(NCORES)))
    return assemble(res.results, Wv, bv, Wf, bf)



# revision 6
# speedup vs baseline: 1.6427x; 1.6427x over previous
"""Trainium2 Bass kernel for nn_DotAttention (B=4, Tq=Tv=2048, D=1024, 16 heads).

Sharding v2: core c -> (batch-pair bp = c//4, head-group hg = c%4 of 4
heads / 256 att dims). Batches are sorted by per-batch kv chunk count
NJ_b = ceil(len/128) and paired so slot widths are (N1, N2) =
(largest, 3rd largest) — each core processes 2 batches with per-batch
chunk counts instead of the global max for all.

Each core computes q/k/v projections for its 256 att-dim slice over its
2 batches, masked softmax attention in transposed-energy layout, and a
partial final projection with its 256-row slice of Wf. Host sums the 4
head-group partials per batch and adds the bias constant (bv @ Wf + bf).

Layouts (SBUF is [128 partitions, free]):
  qT/ctxT [128, 2, 4096]  partition+chunk m = att-dim slice, free = time
                          (2 batches of 2048 side by side)
  kT      [128, 2, NV]    NV = (N1+N2)*128 kv positions
  v       [128, NJ, 260]  partition = kv pos, per head 65 cols (64 v + ones)
  energy^T in PSUM [128(kv), 2*512] head pair side by side
  ctx^T accum in PSUM [65, 512] per head; row 64 = softmax denominator

All matmuls run in fp16 (full PE rate at any free size); exp activations
output bf16; accumulation is fp32 in PSUM. The softmax denominator
reciprocal is broadcast across 64 partitions with a ones[1,64] matmul.
"""

import sys

sys.path.insert(0, "/opt/trn_rl_repo")

import numpy as np

import concourse.bacc as bacc
import concourse.tile as tile
import concourse.mybir as mybir
from concourse.bass_utils import run_bass_kernel_spmd

F32 = mybir.dt.float32
F32R = mybir.dt.float32r
BF16 = mybir.dt.bfloat16
F16 = mybir.dt.float16
MMDT = F16
MM_NP = np.float16
EXDT = BF16
AF = mybir.ActivationFunctionType

B, T, D, ATT = 4, 2048, 1024, 1024
NH, DH = 16, 64
HPC = 4   # heads per core
CD = 256  # att-dim slice per core
NCORES = 8
LARGE = 1e30
SW = 512  # time-span width per streamed input chunk

_cache = {}


def build_nc(cfg, phases="ABC", loop_n=1, ebufs=2, exbufs=3, pcybufs=4):
    N1, N2 = cfg
    key = (N1, N2, phases, loop_n, ebufs, exbufs, pcybufs)
    if key in _cache:
        return _cache[key]
    NJ = N1 + N2           # total kv chunks per core (2 batch slots)
    NV = NJ * 128          # kv positions materialized in kT
    NSV = (NV + SW - 1) // SW  # kv spans (<= 8 always)
    TQ = 2 * T             # q positions per core (2 batches)
    NSQ = TQ // SW         # 8 q spans
    nc = bacc.Bacc("TRN2", target_bir_lowering=False, debug=False,
                   num_devices=NCORES)

    xq_d = nc.dram_tensor("xq", [D, TQ], MMDT, kind="ExternalInput")
    xv_d = nc.dram_tensor("xv", [D, NV], MMDT, kind="ExternalInput")
    wq_d = nc.dram_tensor("wq", [D, CD], MMDT, kind="ExternalInput")
    wk_d = nc.dram_tensor("wk", [D, CD], MMDT, kind="ExternalInput")
    wv_d = nc.dram_tensor("wv", [D, HPC * 65], MMDT, kind="ExternalInput")
    wf_d = nc.dram_tensor("wf", [CD, ATT], MMDT, kind="ExternalInput")
    mask_d = nc.dram_tensor("mask", [128, NJ], F32, kind="ExternalInput")
    bq_d = nc.dram_tensor("bqc", [128, 2], F32, kind="ExternalInput")
    bk_d = nc.dram_tensor("bkc", [128, 2], F32, kind="ExternalInput")
    y_d = nc.dram_tensor("y", [TQ, ATT], F16, kind="ExternalOutput")

    xq_r = xq_d[:, :].rearrange("(kc p) n -> p kc n", p=128)  # [128, 8, TQ]
    xv_r = xv_d[:, :].rearrange("(kc p) n -> p kc n", p=128)
    wq_r = wq_d[:, :].rearrange("(kc p) m -> p kc m", p=128)  # [128, 8, 256]
    wk_r = wk_d[:, :].rearrange("(kc p) m -> p kc m", p=128)
    wv_r = wv_d[:, :].rearrange("(kc p) m -> p kc m", p=128)  # [128, 8, 260]
    wf_r = wf_d[:, :].rearrange("(kc p) n -> p kc n", p=128)  # [128, 2, 1024]

    with tile.TileContext(nc) as tc:
        from contextlib import ExitStack
        _st = ExitStack()
        if loop_n > 1:
            _st.enter_context(tc.For_i(0, loop_n, 1))
        with _st, tc.tile_pool(name="persist", bufs=1) as persist:
            qT = persist.tile([128, 2, TQ], MMDT)
            kT = persist.tile([128, 2, NV], MMDT)
            v = persist.tile([128, NJ, HPC * 65], MMDT)
            mask = persist.tile([128, NJ], F32)
            bqc = persist.tile([128, 2], F32)
            bkc = persist.tile([128, 2], F32)
            ones = persist.tile([1, 64], F32R)

            def set_ones(dst, src):
                nc.scalar.activation(out=dst, in_=src, func=AF.Identity,
                                     bias=1.0, scale=0.0)

            # ---------------- Phase A: projections ----------------
            with (
                tc.tile_pool(name="wpool", bufs=1) as wpool,
                tc.tile_pool(name="chunks", bufs=3) as chunks,
                tc.tile_pool(name="ppq", bufs=4, space="PSUM") as ppq,
                tc.tile_pool(name="ppv", bufs=2, space="PSUM") as ppv,
            ):
                wq = wpool.tile([128, 8, CD], MMDT)
                wk = wpool.tile([128, 8, CD], MMDT)
                wv = wpool.tile([128, 8, HPC * 65], MMDT)
                for s in (range(NSQ) if "A" in phases else []):
                    sl = slice(s * SW, (s + 1) * SW)
                    if s < NSV:
                        sw_v = min(SW, NV - s * SW)
                        slv = slice(s * SW, s * SW + sw_v)
                        xv_c = chunks.tile([128, 8, SW], MMDT, tag="xc")
                        nc.sync.dma_start(out=xv_c[:, :, 0:sw_v],
                                          in_=xv_r[:, :, slv])
                        if s == 0:
                            # wk/wv split per contraction chunk so the first
                            # projection matmuls start as soon as chunk 0 lands
                            for kc in range(8):
                                nc.sync.dma_start(out=wk[:, kc, :],
                                                  in_=wk_r[:, kc, :])
                            for kc in range(8):
                                nc.sync.dma_start(out=wv[:, kc, :],
                                                  in_=wv_r[:, kc, :])
                            nc.sync.dma_start(out=mask, in_=mask_d[:, :])
                            nc.sync.dma_start(out=bqc, in_=bq_d[:, :])
                            nc.sync.dma_start(out=bkc, in_=bk_d[:, :])
                            set_ones(ones, ones)
                            for kc in range(8):
                                nc.scalar.dma_start(out=wq[:, kc, :],
                                                    in_=wq_r[:, kc, :])
                        # kT columns for this span
                        for m in range(2):
                            ps = ppq.tile([128, SW], F32, tag="qk")
                            for kc in range(8):
                                nc.tensor.matmul(
                                    ps[:, 0:sw_v],
                                    lhsT=wk[:, kc, m * 128:(m + 1) * 128],
                                    rhs=xv_c[:, kc, 0:sw_v],
                                    start=(kc == 0), stop=(kc == 7),
                                )
                            with nc.allow_low_precision(reason="qk store"):
                                nc.vector.tensor_scalar_add(
                                    kT[:, m, slv], ps[:, 0:sw_v],
                                    bkc[:, m:m + 1])
                        # v rows for this span (kv tiles of 128)
                        for jt in range(SW // 128):
                            j = s * (SW // 128) + jt
                            if j >= NJ:
                                continue
                            ps = ppv.tile([128, HPC * 65], F32, tag="v")
                            for kc in range(8):
                                nc.tensor.matmul(
                                    ps[:, :],
                                    lhsT=xv_c[:, kc, jt * 128:(jt + 1) * 128],
                                    rhs=wv[:, kc, :],
                                    start=(kc == 0), stop=(kc == 7),
                                )
                            with nc.allow_low_precision(reason="v store"):
                                nc.vector.tensor_copy(out=v[:, j, :],
                                                      in_=ps[:, :])
                            vj = v[:, j, :].rearrange("p (h x) -> p h x", x=65)
                            set_ones(vj[:, :, 64:65], vj[:, :, 64:65])
                    # qT columns for this span
                    xq_c = chunks.tile([128, 8, SW], MMDT, tag="xc")
                    nc.scalar.dma_start(out=xq_c, in_=xq_r[:, :, sl])
                    for m in range(2):
                        ps = ppq.tile([128, SW], F32, tag="qk")
                        for kc in range(8):
                            nc.tensor.matmul(
                                ps[:, :],
                                lhsT=wq[:, kc, m * 128:(m + 1) * 128],
                                rhs=xq_c[:, kc, :],
                                start=(kc == 0), stop=(kc == 7),
                            )
                        with nc.allow_low_precision(reason="qk store"):
                            nc.vector.tensor_scalar_add(
                                qT[:, m, sl], ps[:, :], bqc[:, m:m + 1])

            # ---------------- Phase B: attention ----------------
            with tc.tile_pool(name="bc_sbuf", bufs=1) as bcp:
                ctxT = bcp.tile([128, 2, TQ], MMDT)
                wf = bcp.tile([128, 2, ATT], MMDT)
                nc.scalar.dma_start(out=wf, in_=wf_r)
                if "Z" in phases:  # timing probe: fill ctxT without attention
                    with nc.allow_low_precision(reason="probe"):
                        for m in range(2):
                            for cc in range(TQ // 512):
                                set_ones(ctxT[:, m, cc * 512:(cc + 1) * 512],
                                         ctxT[:, m, cc * 512:(cc + 1) * 512])
                with (
                    tc.tile_pool(name="expp", bufs=exbufs) as expp,
                    tc.tile_pool(name="workp", bufs=4) as workp,
                    tc.tile_pool(name="yp", bufs=4) as yp,
                    tc.tile_pool(name="pe", bufs=ebufs, space="PSUM") as pe_pool,
                    tc.tile_pool(name="pcy", bufs=pcybufs, space="PSUM") as pcy,
                ):
                    def emit_c_unit(i, n):
                        y_ps = pcy.tile([128, 512], F32, tag="cy",
                                        name=f"y_{i}_{n}")
                        for kc in range(2):
                            nc.tensor.matmul(
                                y_ps[:, :],
                                lhsT=ctxT[:, kc, i * 128:(i + 1) * 128],
                                rhs=wf[:, kc, n * 512:(n + 1) * 512],
                                start=(kc == 0), stop=(kc == 1),
                            )
                        y_sb = yp.tile([128, 512], F16, tag="ysb")
                        with nc.allow_low_precision(reason="y store"):
                            nc.vector.tensor_copy(out=y_sb[:, :], in_=y_ps[:, :])
                        nc.scalar.dma_start(
                            out=y_d[i * 128:(i + 1) * 128,
                                    n * 512:(n + 1) * 512],
                            in_=y_sb[:, :])

                    # C units for finished q blocks are spliced into later
                    # blocks' attention to fill PE idle slots
                    pending = []
                    for bslot in range(2):  # batch slot
                        njb = N1 if bslot == 0 else N2
                        joff = 0 if bslot == 0 else N1
                        qoff = bslot * T
                        for ib in range(4):  # Tq block of 512
                            ibs = slice(qoff + ib * 512, qoff + (ib + 1) * 512)
                            for hp in ([0, 1] if "B" in phases else []):
                                ctxA = pcy.tile([65, 512], F32, tag="cy")
                                ctxB = pcy.tile([65, 512], F32, tag="cy")
                                ctx_ps = (ctxA[:, :], ctxB[:, :])
                                for jj in range(njb):
                                    j = joff + jj
                                    e_ps = pe_pool.tile([128, 1024], F32,
                                                        tag="e")
                                    for hh in range(2):
                                        p0 = hh * 64
                                        nc.tensor.matmul(
                                            e_ps[:, hh * 512:(hh + 1) * 512],
                                            lhsT=kT[p0:p0 + 64, hp,
                                                    j * 128:(j + 1) * 128],
                                            rhs=qT[p0:p0 + 64, hp, ibs],
                                            start=True, stop=True,
                                        )
                                    ex = expp.tile([128, 1024], EXDT, tag="ex")
                                    with nc.allow_low_precision(reason="exp"):
                                        nc.scalar.activation(
                                            out=ex[:, :], in_=e_ps[:, :],
                                            func=AF.Exp,
                                            bias=mask[:, j:j + 1], scale=1.0)
                                    for hh in range(2):
                                        h = hp * 2 + hh
                                        nc.tensor.matmul(
                                            ctx_ps[hh],
                                            lhsT=v[:, j, h * 65:(h + 1) * 65],
                                            rhs=ex[:, hh * 512:(hh + 1) * 512],
                                            start=(jj == 0),
                                            stop=(jj == njb - 1),
                                        )
                                for hh in range(2):
                                    p0 = hh * 64
                                    rs = workp.tile([1, 512], F32R, tag="rs")
                                    with nc.allow_low_precision(
                                            reason="f32r is f32 storage"):
                                        nc.vector.reciprocal(
                                            out=rs[:, :],
                                            in_=ctx_ps[hh][64:65, :])
                                    bc_ps = pcy.tile([64, 512], F32, tag="cy")
                                    nc.tensor.matmul(
                                        bc_ps[:, :], lhsT=ones[:, :],
                                        rhs=rs[:, :], start=True, stop=True)
                                    bc_sb = workp.tile([64, 512], F32,
                                                       tag="bcs")
                                    nc.scalar.activation(out=bc_sb[:, :],
                                                         in_=bc_ps[:, :],
                                                         func=AF.Identity)
                                    with nc.allow_low_precision(
                                            reason="ctx store"):
                                        nc.vector.tensor_mul(
                                            ctxT[p0:p0 + 64, hp, ibs],
                                            ctx_ps[hh][0:64, :], bc_sb[:, :],
                                        )
                                for _ in range(2):
                                    if pending:
                                        emit_c_unit(*pending.pop(0))
                            if "C" in phases:
                                base = qoff // 128 + ib * 4
                                pending += [(base + i, n) for i in range(4)
                                            for n in range(2)]
                    while pending:
                        emit_c_unit(*pending.pop(0))
    nc.compile()
    _cache[key] = nc
    return nc


def _plan(value_lens):
    effL = [int(l) if l > 0 else T for l in value_lens]
    NJb = [max(1, int(np.ceil(effL[b] / 128))) for b in range(B)]
    order = list(np.argsort([-n for n in NJb], kind="stable"))
    slots = [[order[0], order[2]], [order[1], order[3]]]  # [bp][slot]
    N1, N2 = NJb[order[0]], NJb[order[2]]
    return slots, N1, N2


def make_in_maps(query, value, value_lens, Wq, bq, Wk, bk, Wv, bv, Wf, bf):
    query = np.ascontiguousarray(np.asarray(query, np.float32))
    value = np.ascontiguousarray(np.asarray(value, np.float32))
    value_lens = np.asarray(value_lens)
    Wq = np.asarray(Wq, np.float32)
    Wk = np.asarray(Wk, np.float32)
    Wv = np.asarray(Wv, np.float32)
    Wf = np.asarray(Wf, np.float32)
    bq = np.asarray(bq, np.float32)
    bk = np.asarray(bk, np.float32)

    scale = 1.0 / np.sqrt(np.float32(DH))
    slots, N1, N2 = _plan(value_lens)
    widths = (N1, N2)

    in_maps = []
    for c in range(NCORES):
        bp, hg = c // 4, c % 4
        bA, bB = slots[bp]
        cs = slice(hg * CD, (hg + 1) * CD)
        xq = np.empty((D, 2 * T), np.float32)
        for si, b in enumerate((bA, bB)):
            xq[:, si * T:(si + 1) * T] = (
                0.0 if value_lens[b] == 0 else query[b].T)
        xv = np.concatenate([value[bA].T[:, :N1 * 128],
                             value[bB].T[:, :N2 * 128]], axis=1)
        wq = (Wq[:, cs] * scale).copy()
        wk = Wk[:, cs].copy()
        wv = np.zeros((D, HPC * 65), np.float32)
        for h in range(HPC):
            wv[:, h * 65:h * 65 + 64] = \
                Wv[:, hg * CD + h * 64:hg * CD + (h + 1) * 64]
        wf = Wf[cs, :].copy()
        mask = np.zeros((128, N1 + N2), np.float32)
        for si, b in enumerate((bA, bB)):
            L = int(value_lens[b])
            joff, nw = (0, N1) if si == 0 else (N1, N2)
            if L > 0:
                idx = np.arange(nw * 128).reshape(nw, 128).T  # [128, nw]
                msl = mask[:, joff:joff + nw]
                msl[idx >= L] = -LARGE
        bqc = (bq[cs] * scale).reshape(2, 128).T.copy()
        bkc = bk[cs].reshape(2, 128).T.copy()
        in_maps.append({
            "xq": xq.astype(MM_NP), "xv": xv.astype(MM_NP),
            "wq": wq.astype(MM_NP), "wk": wk.astype(MM_NP),
            "wv": wv.astype(MM_NP), "wf": wf.astype(MM_NP),
            "mask": mask, "bqc": bqc, "bkc": bkc,
        })
    return in_maps, widths


def assemble(results, value_lens, Wv, bv, Wf, bf):
    bv = np.asarray(bv, np.float32)
    Wf = np.asarray(Wf, np.float32)
    bf = np.asarray(bf, np.float32)
    slots, N1, N2 = _plan(np.asarray(value_lens))
    out = np.empty((B, T, ATT), np.float32)
    const = (bv @ Wf + bf).astype(np.float32)
    for bp in range(2):
        for si, b in enumerate(slots[bp]):
            acc = results[bp * 4]["y"][si * T:(si + 1) * T].astype(np.float32)
            for hg in range(1, 4):
                acc += results[bp * 4 + hg]["y"][si * T:(si + 1) * T]
            out[b] = acc + const
    return out


def kernel(query, value, value_lens, Wq, bq, Wk, bk, Wv, bv, Wf, bf):
    in_maps, widths = make_in_maps(query, value, value_lens, Wq, bq, Wk, bk,
                                   Wv, bv, Wf, bf)
    nc = build_nc(widths)
    res = run_bass_kernel_spmd(nc, in_maps, list(range(NCORES)))
    return assemble(res.results, value_lens, Wv, bv, Wf, bf)


# revision 57
# speedup vs baseline: 1.9997x; 1.2173x over previous
"""Trainium2 Bass kernel for nn_DotAttention (B=4, Tq=Tv=2048, D=1024, 16 heads).

Sharding: core c -> (batch-pair bp = c//4, head-group hg = c%4 of 4
heads / 256 att dims). Batches are sorted by per-batch kv chunk count
NJ_b = ceil(len/128) and paired so compiled slot widths are (N1, N2) =
(largest, 3rd-largest); each core runs 2 batch slots with those widths.

Each core computes q/k/v projections for its 256 att-dim slice over its
2 batches, masked softmax attention in transposed-energy layout, and a
partial final projection with its 256-row slice of Wf. Host sums the 4
head-group partials per batch and adds the bias constant (bv @ Wf + bf).

Structure: the For_i body holds TWO software-pipeline halves with
alternating SBUF tile sets. While half h runs attention + final proj
from set[1-h], the projections (phase A) for set[h] are interleaved
into its ACT-bound j-slots, keeping the PE busy. Iteration 0's first
half consumes uninitialized tiles; its (garbage) y rows are overwritten
by the correct second half, so single-shot runs stay correct.

Masking lives in v: masked kv rows and their ones-column entries are
zeroed (host-built 0/1 masks), so the exp needs no per-chunk bias and
exps batch two kv chunks [128, 1024] per instruction.

The softmax denominator (row 64 of each ctx accumulator) is
reciprocated with the fast-approx DVE op and broadcast across 64
partitions by a DRAM round-trip DMA on the gpsimd queue; the DVE mul
is deferred until the broadcast has landed (age-based scheduling).
"""

import sys

sys.path.insert(0, "/opt/trn_rl_repo")

import numpy as np

import concourse.bacc as bacc
import concourse.tile as tile
import concourse.mybir as mybir
from concourse.bass_utils import run_bass_kernel_spmd

F32 = mybir.dt.float32
F32R = mybir.dt.float32r
BF16 = mybir.dt.bfloat16
F16 = mybir.dt.float16
MMDT = F16
MM_NP = np.float16
EXDT = BF16
AF = mybir.ActivationFunctionType

B, T, D, ATT = 4, 2048, 1024, 1024
NH, DH = 16, 64
HPC = 4   # heads per core
CD = 256  # att-dim slice per core
NCORES = 8
LARGE = 1e30
SWA = 1024  # input span width (2KB DMA lines)

_cache = {}


def build_nc(cfg, phases="ABC", loop_n=1, ebufs=2, exbufs=5, halves=1):
    N1, N2 = cfg
    key = (N1, N2, phases, loop_n, ebufs, exbufs, halves)
    if key in _cache:
        return _cache[key]
    NJ = N1 + N2           # total kv chunks per core (2 batch slots)
    NV = NJ * 128          # kv positions materialized in kT
    TQ = 2 * T             # q positions per core (2 batches)
    NSQA = TQ // SWA       # 4 q spans
    nc = bacc.Bacc("TRN2", target_bir_lowering=False, debug=False,
                   num_devices=NCORES)

    xq_d = nc.dram_tensor("xq", [D, TQ], MMDT, kind="ExternalInput")
    xv_d = nc.dram_tensor("xv", [D, NV], MMDT, kind="ExternalInput")
    wq_d = nc.dram_tensor("wq", [D, CD], MMDT, kind="ExternalInput")
    wk_d = nc.dram_tensor("wk", [D, CD], MMDT, kind="ExternalInput")
    wv_d = nc.dram_tensor("wv", [D, HPC * 65], MMDT, kind="ExternalInput")
    wf_d = nc.dram_tensor("wf", [CD, ATT], MMDT, kind="ExternalInput")
    maskv_d = nc.dram_tensor("maskv", [128, NJ], F32, kind="ExternalInput")
    maskv4_d = nc.dram_tensor("maskv4", [128, NJ * HPC], MMDT,
                              kind="ExternalInput")
    bq_d = nc.dram_tensor("bqc", [128, 2], F32, kind="ExternalInput")
    bk_d = nc.dram_tensor("bkc", [128, 2], F32, kind="ExternalInput")
    y_d = nc.dram_tensor("y", [TQ, ATT], F16, kind="ExternalOutput")

    xq_r = xq_d[:, :].rearrange("(kc p) n -> p kc n", p=128)  # [128, 8, TQ]
    xv_r = xv_d[:, :].rearrange("(kc p) n -> p kc n", p=128)
    wq_r = wq_d[:, :].rearrange("(kc p) m -> p kc m", p=128)  # [128, 8, 256]
    wk_r = wk_d[:, :].rearrange("(kc p) m -> p kc m", p=128)
    wv_r = wv_d[:, :].rearrange("(kc p) m -> p kc m", p=128)  # [128, 8, 260]
    wf_r = wf_d[:, :].rearrange("(kc p) n -> p kc n", p=128)  # [128, 2, 1024]

    with tile.TileContext(nc) as tc:
        from contextlib import ExitStack
        _st = ExitStack()
        if loop_n > 1:
            _st.enter_context(tc.For_i(0, loop_n, 1))
        with _st, tc.tile_pool(name="persist", bufs=1) as persist, \
                tc.tile_pool(name="chunks", bufs=3) as chunks, \
                tc.tile_pool(name="expp", bufs=exbufs) as expp, \
                tc.tile_pool(name="workp", bufs=4) as workp, \
                tc.tile_pool(name="yp", bufs=3) as yp, \
                tc.tile_pool(name="pe", bufs=ebufs, space="PSUM") as pe_pool, \
                tc.tile_pool(name="pctx", bufs=3, space="PSUM") as pctx, \
                tc.tile_pool(name="py", bufs=1, space="PSUM") as py, \
                tc.tile_pool(name="rsd", bufs=4, space="DRAM") as rsd:
            sets = []
            for si in range(2):
                sets.append(dict(
                    qT=persist.tile([128, 2, TQ], MMDT, name=f"qT{si}"),
                    kT=persist.tile([128, 2, NV], MMDT, name=f"kT{si}"),
                    v=persist.tile([128, NJ, HPC * 65], MMDT, name=f"v{si}"),
                    ctxT=persist.tile([128, 2, TQ], MMDT, name=f"ctxT{si}"),
                ))
            maskv = persist.tile([128, NJ], F32)
            maskv4 = persist.tile([128, NJ, HPC], MMDT)
            bqc = persist.tile([128, 2], F32)
            bkc = persist.tile([128, 2], F32)
            wq = persist.tile([128, 8, CD], MMDT)
            wk = persist.tile([128, 8, CD], MMDT)
            wv = persist.tile([128, 8, HPC * 65], MMDT)
            wf = persist.tile([128, 2, ATT], MMDT)

            # one-time small loads (identical every iteration/half)
            nc.gpsimd.dma_start(out=bkc, in_=bk_d[:, :])
            nc.gpsimd.dma_start(out=bqc, in_=bq_d[:, :])
            nc.gpsimd.dma_start(out=maskv, in_=maskv_d[:, :])
            nc.gpsimd.dma_start(
                out=maskv4,
                in_=maskv4_d[:, :].rearrange("p (j h) -> p j h", h=HPC))
            nc.sync.dma_start(out=wk, in_=wk_r)
            nc.sync.dma_start(out=wv, in_=wv_r)
            nc.sync.dma_start(out=wq, in_=wq_r)
            nc.sync.dma_start(out=wf, in_=wf_r)

            # ---------- phase A work-unit factory (fills one set) ----------
            # A PSUM tiles come from the shared energy ring (tag "e"),
            # so A+B together stay inside the 8 PSUM banks.
            def a_units(st):
                """Returns span closures; each expands to ~1-PE-matmul-group
                sub-closures when popped (DMA is emitted at expansion)."""
                units = []
                qT, kT, v = st["qT"], st["kT"], st["v"]

                def load_span(r_view, width, off, tag):
                    cch = chunks.tile([128, 8, SWA], MMDT, tag="xc",
                                      name=f"xc_{tag}")
                    nc.sync.dma_start(out=cch[:, :, 0:width],
                                      in_=r_view[:, :, off:off + width])
                    return cch

                # kv spans first (attention consumes kT/v from chunk 0)
                ns_v = (NV + SWA - 1) // SWA
                for s in range(ns_v):
                    sw_v = min(SWA, NV - s * SWA)

                    def kv_span(s=s, sw_v=sw_v):
                        cch = load_span(xv_r, sw_v, s * SWA, f"v{s}")

                        def kmm(mh, cch=cch, sw_v=sw_v, s=s):
                            m, hw0 = mh // 2, (mh % 2) * 512
                            hw1 = min(hw0 + 512, sw_v)
                            if hw0 >= sw_v:
                                return
                            ps = pe_pool.tile([128, 512], F32, tag="e",
                                              name=f"kps{s}_{mh}")
                            for kc in range(8):
                                nc.tensor.matmul(
                                    ps[:, 0:hw1 - hw0],
                                    lhsT=wk[:, kc, m * 128:(m + 1) * 128],
                                    rhs=cch[:, kc, hw0:hw1],
                                    start=(kc == 0), stop=(kc == 7),
                                )
                            with nc.allow_low_precision(reason="qk store"):
                                nc.vector.tensor_scalar_add(
                                    kT[:, m, s * SWA + hw0:s * SWA + hw1],
                                    ps[:, 0:hw1 - hw0], bkc[:, m:m + 1])

                        def vmm(jt, cch=cch, s=s):
                            j = s * (SWA // 128) + jt
                            if j >= NJ:
                                return
                            ps = pe_pool.tile([128, HPC * 65], F32, tag="e",
                                              name=f"vps{j}")
                            for kc in range(8):
                                nc.tensor.matmul(
                                    ps[:, 0:HPC * 65],
                                    lhsT=cch[:, kc,
                                             jt * 128:(jt + 1) * 128],
                                    rhs=wv[:, kc, :],
                                    start=(kc == 0), stop=(kc == 7),
                                )
                            with nc.allow_low_precision(reason="v store"):
                                nc.vector.tensor_scalar_mul(
                                    v[:, j, :], ps[:, 0:HPC * 65],
                                    maskv[:, j:j + 1])
                            vj = v[:, j, :].rearrange(
                                "p (h x) -> p h x", x=65)
                            nc.gpsimd.tensor_copy(out=vj[:, :, 64:65],
                                                  in_=maskv4[:, j, :])
                        nv_t = min(SWA // 128, NJ - s * (SWA // 128))
                        return [lambda mh=mh: kmm(mh) for mh in range(4)] + \
                               [lambda t=t: vmm(t) for t in range(nv_t)]
                    units.append(kv_span)

                for s in range(NSQA):
                    def q_span(s=s):
                        cch = load_span(xq_r, SWA, s * SWA, f"q{s}")

                        def qmm(mh, cch=cch, s=s):
                            m, hw0 = mh // 2, (mh % 2) * 512
                            ps = pe_pool.tile([128, 512], F32, tag="e",
                                              name=f"qps{s}_{mh}")
                            for kc in range(8):
                                nc.tensor.matmul(
                                    ps[:, :],
                                    lhsT=wq[:, kc, m * 128:(m + 1) * 128],
                                    rhs=cch[:, kc, hw0:hw0 + 512],
                                    start=(kc == 0), stop=(kc == 7),
                                )
                            with nc.allow_low_precision(reason="qk store"):
                                nc.vector.tensor_scalar_add(
                                    qT[:, m,
                                       s * SWA + hw0:s * SWA + hw0 + 512],
                                    ps[:, :], bqc[:, m:m + 1])
                        return [lambda mh=mh: qmm(mh) for mh in range(4)]
                    units.append(q_span)
                return units

            # ----------------- per-half emission ------------------------
            def emit_half(src, dst, prefill=False):
                """Attention+final-proj from src set; phase A into dst set.
                prefill=True runs all A work before attention (single-half
                mode: src is dst)."""
                qT, kT, v, ctxT = (src["qT"], src["kT"], src["v"],
                                   src["ctxT"])
                tick = [0]
                norms = []      # (ready, fn) gate ctx-ring reuse
                cwork = []      # (ready, fn) final-proj units
                delay = []      # delayed ctx matmuls (2-slot trail)
                aq = list(a_units(dst)) if "A" in phases else []
                apend = []      # expanded A sub-units
                if prefill:
                    while aq or apend:
                        if not apend and aq:
                            apend.extend(aq.pop(0)())
                        while apend:
                            apend.pop(0)()

                def emit_c_half(i, n, y_sb):
                    y_ps = py.tile([128, 512], F32, tag="cy",
                                   name=f"y_{i}_{n}")
                    for kc in range(2):
                        nc.tensor.matmul(
                            y_ps[:, :],
                            lhsT=ctxT[:, kc, i * 128:(i + 1) * 128],
                            rhs=wf[:, kc, n * 512:(n + 1) * 512],
                            start=(kc == 0), stop=(kc == 1),
                        )
                    with nc.allow_low_precision(reason="y store"):
                        nc.vector.tensor_copy(
                            out=y_sb[:, n * 512:(n + 1) * 512],
                            in_=y_ps[:, :])
                    if n == 1:
                        nc.gpsimd.dma_start(
                            out=y_d[i * 128:(i + 1) * 128, :],
                            in_=y_sb[:, :])

                def emit_c_unit(i):
                    y_sb = yp.tile([128, ATT], F16, tag="ysb",
                                   name=f"ysb_{i}")
                    cwork.append((tick[0] + 2,
                                  lambda: emit_c_half(i, 0, y_sb)))
                    cwork.append((tick[0] + 2,
                                  lambda: emit_c_half(i, 1, y_sb)))

                n2done = [0]

                def make_norm(ctx, rs, p0, m, qsl, c_base):
                    def norm2(bc_sb):
                        with nc.allow_low_precision(reason="ctx store"):
                            nc.vector.tensor_mul(
                                ctxT[p0:p0 + 64, m, qsl],
                                ctx[0:64, :], bc_sb[:, :])
                        n2done[0] += 1
                        if c_base is not None and "C" in phases:
                            for i in range(4):
                                emit_c_unit(c_base + i)

                    def norm1():
                        # broadcast via DRAM round-trip (SP queue: the SWDGE
                        # path mishandles partition_broadcast)
                        rs_dr = rsd.tile([1, 512], F32, tag="rsd")
                        nc.sync.dma_start(out=rs_dr[:, :], in_=rs[:, :])
                        bc_sb = workp.tile([64, 512], F32, tag="bcs")
                        nc.sync.dma_start(
                            out=bc_sb[:, :],
                            in_=rs_dr[0:1, :].partition_broadcast(64))
                        norms.append((tick[0] + 4, lambda: norm2(bc_sb)))
                    return norm1

                def pop_a():
                    if not apend and aq:
                        apend.extend(aq.pop(0)())
                    if apend:
                        apend.pop(0)()
                        return True
                    return False

                def pop_deferred():
                    if norms and norms[0][0] <= tick[0]:
                        norms.pop(0)[1]()
                    elif cwork and cwork[0][0] <= tick[0]:
                        cwork.pop(0)[1]()
                    else:
                        pop_a()

                groups = [(bslot, ib, h)
                          for bslot in range(2)
                          for ib in range(4)
                          for h in (range(HPC) if "B" in phases else [])]
                for gi, (bslot, ib, h) in enumerate(groups):
                    njb = N1 if bslot == 0 else N2
                    joff = 0 if bslot == 0 else N1
                    qoff = bslot * T + ib * 512
                    ibs = slice(qoff, qoff + 512)
                    p0, m = (h % 2) * 64, h // 2
                    # the ctx ring has 3 buffers: before reusing the tile
                    # from 3 groups ago its deferred mul MUST have been
                    # emitted, else the tile framework misses the reader
                    while n2done[0] < gi - 2 and norms:
                        norms.pop(0)[1]()
                    ctx = pctx.tile([65, 512], F32, tag="ctx")
                    npair = (njb + 1) // 2
                    for pp in range(npair):
                        ja = joff + 2 * pp
                        jb = ja + 1 if 2 * pp + 1 < njb else None
                        wex = 1024 if jb is not None else 512
                        e_ps = pe_pool.tile([128, 1024], F32, tag="e")
                        nc.tensor.matmul(
                            e_ps[:, 0:512],
                            lhsT=kT[p0:p0 + 64, m, ja * 128:(ja + 1) * 128],
                            rhs=qT[p0:p0 + 64, m, ibs],
                            start=True, stop=True,
                        )
                        if jb is not None:
                            nc.tensor.matmul(
                                e_ps[:, 512:1024],
                                lhsT=kT[p0:p0 + 64, m,
                                        jb * 128:(jb + 1) * 128],
                                rhs=qT[p0:p0 + 64, m, ibs],
                                start=True, stop=True,
                            )
                        ex = expp.tile([128, 1024], EXDT, tag="ex")
                        with nc.allow_low_precision(reason="exp"):
                            nc.scalar.activation(
                                out=ex[:, 0:wex], in_=e_ps[:, 0:wex],
                                func=AF.Exp)

                        def emit_ctx(pp=pp, ja=ja, jb=jb, ex=ex, ctx=ctx,
                                     h=h, npair=npair, qoff=qoff,
                                     p0=p0, m=m):
                            nc.tensor.matmul(
                                ctx[:, :],
                                lhsT=v[:, ja, h * 65:(h + 1) * 65],
                                rhs=ex[:, 0:512],
                                start=(pp == 0),
                                stop=(jb is None and pp == npair - 1),
                            )
                            if jb is not None:
                                nc.tensor.matmul(
                                    ctx[:, :],
                                    lhsT=v[:, jb, h * 65:(h + 1) * 65],
                                    rhs=ex[:, 512:1024],
                                    start=False, stop=(pp == npair - 1),
                                )
                            if pp == npair - 1:
                                last = (h == HPC - 1)
                                qsl = slice(qoff, qoff + 512)
                                c_base = qoff // 128 if last else None
                                den = workp.tile([1, 512], F32,
                                                 tag="den", name="den")
                                nc.vector.tensor_copy(out=den[:, :],
                                                      in_=ctx[64:65, :])
                                rs = workp.tile([1, 512], F32,
                                                tag="rs", name="rs")
                                nc.vector.reciprocal_approx_fast(
                                    out=rs[:, :], in_=den[:, :])
                                norms.append((tick[0] + 2, make_norm(
                                    ctx, rs, p0, m, qsl, c_base)))
                        delay.append(emit_ctx)
                        tick[0] += 1
                        while len(delay) > 2:
                            delay.pop(0)()
                        pop_deferred()
                while delay:
                    delay.pop(0)()
                while norms or cwork or aq or apend:
                    tick[0] += 100
                    if norms:
                        norms.pop(0)[1]()
                    elif cwork:
                        cwork.pop(0)[1]()
                    elif not pop_a():
                        break
                if "B" not in phases and "C" in phases:
                    for i in range(TQ // 128):
                        y_sb = yp.tile([128, ATT], F16, tag="ysb",
                                       name=f"ysbz_{i}")
                        emit_c_half(i, 0, y_sb)
                        emit_c_half(i, 1, y_sb)
                if "Z" in phases:
                    with nc.allow_low_precision(reason="probe"):
                        for mm in range(2):
                            for cc in range(TQ // 512):
                                nc.scalar.activation(
                                    out=ctxT[:, mm, cc * 512:(cc + 1) * 512],
                                    in_=ctxT[:, mm, cc * 512:(cc + 1) * 512],
                                    func=AF.Identity, bias=1.0, scale=0.0)

            if halves == 1:
                emit_half(sets[0], sets[0], prefill=True)
            else:
                emit_half(sets[1], sets[0])
                emit_half(sets[0], sets[1])
    nc.compile()
    _cache[key] = nc
    return nc


def _plan(value_lens):
    effL = [int(l) if l > 0 else T for l in value_lens]
    NJb = [max(1, int(np.ceil(effL[b] / 128))) for b in range(B)]
    order = list(np.argsort([-n for n in NJb], kind="stable"))
    slots = [[order[0], order[2]], [order[1], order[3]]]  # [bp][slot]
    N1, N2 = NJb[order[0]], NJb[order[2]]
    return slots, N1, N2


def make_in_maps(query, value, value_lens, Wq, bq, Wk, bk, Wv, bv, Wf, bf):
    query = np.ascontiguousarray(np.asarray(query, np.float32))
    value = np.ascontiguousarray(np.asarray(value, np.float32))
    value_lens = np.asarray(value_lens)
    Wq = np.asarray(Wq, np.float32)
    Wk = np.asarray(Wk, np.float32)
    Wv = np.asarray(Wv, np.float32)
    Wf = np.asarray(Wf, np.float32)
    bq = np.asarray(bq, np.float32)
    bk = np.asarray(bk, np.float32)

    scale = 1.0 / np.sqrt(np.float32(DH))
    slots, N1, N2 = _plan(value_lens)
    widths = (N1, N2)

    in_maps = []
    for c in range(NCORES):
        bp, hg = c // 4, c % 4
        bA, bB = slots[bp]
        cs = slice(hg * CD, (hg + 1) * CD)
        xq = np.empty((D, 2 * T), np.float32)
        for si, b in enumerate((bA, bB)):
            xq[:, si * T:(si + 1) * T] = (
                0.0 if value_lens[b] == 0 else query[b].T)
        xv = np.concatenate([value[bA].T[:, :N1 * 128],
                             value[bB].T[:, :N2 * 128]], axis=1)
        wq = (Wq[:, cs] * scale).copy()
        wk = Wk[:, cs].copy()
        wv = np.zeros((D, HPC * 65), np.float32)
        for h in range(HPC):
            wv[:, h * 65:h * 65 + 64] = \
                Wv[:, hg * CD + h * 64:hg * CD + (h + 1) * 64]
        wf = Wf[cs, :].copy()
        maskv = np.ones((128, N1 + N2), np.float32)
        for si, b in enumerate((bA, bB)):
            L = int(value_lens[b])
            joff, nw = (0, N1) if si == 0 else (N1, N2)
            if L > 0:
                idx = np.arange(nw * 128).reshape(nw, 128).T  # [128, nw]
                msl = maskv[:, joff:joff + nw]
                msl[idx >= L] = 0.0
        maskv4 = np.repeat(maskv[:, :, None], HPC, axis=2).reshape(
            128, (N1 + N2) * HPC)
        bqc = (bq[cs] * scale).reshape(2, 128).T.copy()
        bkc = bk[cs].reshape(2, 128).T.copy()
        in_maps.append({
            "xq": xq.astype(MM_NP), "xv": xv.astype(MM_NP),
            "wq": wq.astype(MM_NP), "wk": wk.astype(MM_NP),
            "wv": wv.astype(MM_NP), "wf": wf.astype(MM_NP),
            "maskv": maskv, "maskv4": maskv4.astype(MM_NP),
            "bqc": bqc, "bkc": bkc,
        })
    return in_maps, widths


def assemble(results, value_lens, Wv, bv, Wf, bf):
    bv = np.asarray(bv, np.float32)
    Wf = np.asarray(Wf, np.float32)
    bf = np.asarray(bf, np.float32)
    slots, N1, N2 = _plan(np.asarray(value_lens))
    out = np.empty((B, T, ATT), np.float32)
    const = (bv @ Wf + bf).astype(np.float32)
    for bp in range(2):
        for si, b in enumerate(slots[bp]):
            acc = results[bp * 4]["y"][si * T:(si + 1) * T].astype(np.float32)
            for hg in range(1, 4):
                acc += results[bp * 4 + hg]["y"][si * T:(si + 1) * T]
            out[b] = acc + const
    return out


def kernel(query, value, value_lens, Wq, bq, Wk, bk, Wv, bv, Wf, bf):
    in_maps, widths = make_in_maps(query, value, value_lens, Wq, bq, Wk, bk,
                                   Wv, bv, Wf, bf)
    nc = build_nc(widths)
    res = run_bass_kernel_spmd(nc, in_maps, list(range(NCORES)))
    return assemble(res.results, value_lens, Wv, bv, Wf, bf)
